# revision 5
# baseline (speedup 1.0000x reference)
"""DirectionalRotationLoss Trainium2 kernel (8-core data-parallel), v3.

Math (per quaternion pair p, t — both unnormalized, q = (w,x,y,z)):
  With s1 = (w^2+z^2)/2, s2 = (x^2+y^2)/2, u3 = s1-s2, n2 = s1+s2,
  u1 = wy+xz, u2 = yz-wx the vector u = (u1,u2,u3) satisfies |u| = n2 and
  u/n2 = R(q_hat) e3 up to a fixed diagonal scaling that cancels between
  pred and target.  d = <u_p, u_t>, g = n2_p * n2_t.
  theta = 2*atan(sqrt((g-d)/(g+d))) — tangent half-angle, no arccos.
  With m1 = max(min(g-d, g+d), tiny), m2 = max(g-d, g+d),
  phi = atan(sqrt(m1/m2)):  theta^2/4 = phi^2 + [d<0]*(pi^2/4 - pi*phi).
  Mean over all pairs (x4 applied on host).

Layout: the host packs BOTH tensors into one DRAM param per core with
  per-iteration blocks [w_p w_t | x_p x_t | y_p y_t | z_p z_t], each [P, M].
  One SWDGE cast-DMA per iteration (8 MiB, f32 read -> bf16 in SBUF).
  Component-pair-major order makes every batched operand below a contiguous
  slice, so all stock tensor_tensor ops run in 2x bf16 mode.

Engine split per iteration (tile TT = [P, 8M] bf16):
  GpSimd:  cast-DMA; yz/wx product pair (POOL_OFFLOAD).
  VectorE: products/adds in batched 2M/4M instructions; custom DVE ops
           M1CLAMP, MAXRECIP, ATAN_ODD7, THETA2ACC (theta^2/4 + accumulate).
  ScalarE: 4 in-place Squares over TT (scale=1/sqrt2) after the products
           have consumed TT, plus the Sqrt — one ACT table set.
"""

import numpy as np
from operator import add as _op_add

import concourse.bass as bass
import concourse.bacc as bacc
import concourse.mybir as mybir
from concourse.tile import TileContext
from concourse.bass_utils import run_bass_kernel_spmd
from concourse.dve_spec import (
    Spec,
    Src0,
    Src1,
    C0,
    C1,
    C2,
    Zero,
    One,
    lower,
    sq,
    _has_src1,
    Bin,
    AluOp as _AluOp,
    maxx as _maxx,
    minn as _minn,
)
from concourse.dve_uop import DveOpSpec
import concourse.dve_ops as dve_ops
from concourse.dve_ops import (
    DveOp,
    OPS,
    get_dve_sub_opcode,
    RECIP_APPROX_FAST_CONSTS,
)

NCORES = 8
P = 128
B = 8388608
QPC = B // NCORES          # quats per core
QPP = QPC // P             # quats per partition (8192)
M = 2048                   # quats per partition per iteration
NIT = QPP // M             # iterations (4)

F32 = mybir.dt.float32
BF16 = mybir.dt.bfloat16
AL = mybir.AluOpType
AF = mybir.ActivationFunctionType

INV_SQRT2 = float(1.0 / np.sqrt(2.0))
TINY = 1e-6
PI = float(np.pi)

# 2 -> yz & wx product instructions run on GpSimd instead of VectorE.
POOL_OFFLOAD = 2

# atan(t) ~= t*(1 + t^2*(A1 + A2 t^2 + A3 t^4)) on [0,1]; max err ~8.8e-4 rad.
A1, A2, A3 = -0.33215468080413524, 0.17587160443286803, -0.059202957570287394


def _make_op(name, spec, subdim=False):
    for op in OPS:
        if op.name == name:
            return op
    shas = {}
    op = DveOp(name, spec, subdim=subdim, uops_sha=shas)
    OPS.append(op)
    dve_ops.CUSTOM_DVE_SPECS[name] = spec
    dve_ops._SUB_OPCODE_FOR_NAME[name] = dve_ops._CUSTOM_DVE_ROW_BASE + len(OPS) - 1
    for ver in ("v3", "v4"):
        r = DveOpSpec(
            name=name,
            opcode=get_dve_sub_opcode(name),
            uops=lower(spec, ver=ver),
            rd1_en=_has_src1(spec),
        )
        shas[ver] = r.sha(ver)
    return op


_t2 = sq(Src0)
ATAN7 = _make_op(
    "ATAN_ODD7_ANT",
    Spec(
        body=(((C2 * _t2 + C1) * _t2 + C0) * _t2 + One) * Src0,
        reference=lambda in0, in1, s0, s1, imm2: (
            ((imm2 * in0 * in0 + s1) * in0 * in0 + s0) * in0 * in0 + 1.0
        )
        * in0,
    ),
)

_sub = Src0 - Src1
_add2 = Src0 + Src1
M1C = _make_op(
    "M1CLAMP_ANT",
    Spec(
        body=_maxx(_minn(_sub, _add2), C0),
        reference=lambda in0, in1, s0, s1, imm2: np.maximum(
            np.minimum(in0 - in1, in0 + in1), s0
        ),
    ),
)

# MAXRECIP: y ~= 1/max(in0-in1, in0+in1) via bitwise-NOT seed + 1 Newton step.
_m2 = _maxx(_sub, _add2)
_nt = Bin(_AluOp.BITWISE_NOT, _m2, _m2)
_y0 = _nt * C0
_y1 = _y0 * (C1 - _m2 * _y0)
M2R = _make_op(
    "MAXRECIP_ANT",
    Spec(
        body=_y1,
        reference=lambda in0, in1, s0, s1, imm2: (
            lambda m: (lambda y0: y0 * (s1 - m * y0))(
                (~m.view(np.int32)).view(np.float32) * s0
            )
        )(np.maximum(in0 - in1, in0 + in1)),
    ),
)

# THETA2ACC: out = phi^2 + [d<0]*(C0 - C1*phi); accum_out = sum over free dim.
THETA2 = _make_op(
    "THETA2ACC_ANT",
    Spec(
        body=sq(Src1) + (Src0 < Zero) * (C0 - C1 * Src1),
        accum=_op_add,
        accum_init=Zero,
        reference=lambda in0, in1, s0, s1, imm2: in1 * in1
        + (in0 < 0) * (s0 - s1 * in1),
    ),
)


def _emit(nc, reps=1):
    pt = nc.declare_dram_parameter("pt", [P, QPP * 8], F32, isOutput=False)
    out = nc.declare_dram_parameter("out", [P, NIT], F32, isOutput=True)
    rc = RECIP_APPROX_FAST_CONSTS

    with TileContext(nc) as tc:
        with (
            tc.tile_pool(name="stg", bufs=2) as stg,
            tc.tile_pool(name="pp", bufs=1) as ppp,
            tc.tile_pool(name="w2", bufs=6) as w2p,
            tc.tile_pool(name="u", bufs=6) as up,
            tc.tile_pool(name="tl", bufs=7) as tlp,
            tc.tile_pool(name="st", bufs=1) as stp,
        ):
            stats = stp.tile([P, NIT], F32, tag="st", name="stats")

            import contextlib

            loop_cm = tc.For_i(0, reps, 1) if reps > 1 else contextlib.nullcontext()
            with loop_cm:
              for it in range(NIT):
                TT = stg.tile([P, 8 * M], BF16, tag="stage", name=f"TT{it}")
                nc.gpsimd.dma_start(
                    out=TT[:, :], in_=pt[:, it * 8 * M : (it + 1) * 8 * M]
                )
                # block views (each [P, 2M], both sides side-by-side)
                wb = TT[:, 0 : 2 * M]
                xb = TT[:, 2 * M : 4 * M]
                yb = TT[:, 4 * M : 6 * M]
                zb = TT[:, 6 * M : 8 * M]

                # products first (they read raw TT)
                pp = ppp.tile([P, 4 * M], BF16, tag="pp", name=f"pp{it}")
                nc.vector.tensor_mul(pp[:, :], TT[:, 0 : 4 * M], TT[:, 4 * M : 8 * M])
                yz = w2p.tile([P, 2 * M], BF16, tag="w2", name=f"yz{it}")
                wx = w2p.tile([P, 2 * M], BF16, tag="w2", name=f"wx{it}")
                peng = nc.gpsimd if POOL_OFFLOAD >= 2 else nc.vector
                peng.tensor_mul(yz[:, :], yb, zb)
                peng.tensor_mul(wx[:, :], wb, xb)

                u1 = up.tile([P, 2 * M], BF16, tag="u", name=f"u1{it}")
                u2 = up.tile([P, 2 * M], BF16, tag="u", name=f"u2{it}")
                nc.vector.tensor_add(u1[:, :], pp[:, 0 : 2 * M], pp[:, 2 * M : 4 * M])
                nc.vector.tensor_sub(u2[:, :], yz[:, :], wx[:, :])

                # in-place squares (scale folds the 1/2): TT <- (TT/sqrt2)^2
                for blk in (wb, xb, yb, zb):
                    nc.scalar.activation(blk, blk, AF.Square, scale=INV_SQRT2)

                s1 = w2p.tile([P, 2 * M], BF16, tag="w2", name=f"s1{it}")
                s2 = w2p.tile([P, 2 * M], BF16, tag="w2", name=f"s2{it}")
                nc.vector.tensor_add(s1[:, :], wb, zb)
                nc.vector.tensor_add(s2[:, :], xb, yb)
                u3 = up.tile([P, 2 * M], BF16, tag="u", name=f"u3{it}")
                n2 = up.tile([P, 2 * M], BF16, tag="u", name=f"n2{it}")
                nc.vector.tensor_sub(u3[:, :], s1[:, :], s2[:, :])
                nc.vector.tensor_add(n2[:, :], s1[:, :], s2[:, :])

                # combine p/t halves
                d1 = tlp.tile([P, M], BF16, tag="tl", name=f"d1{it}")
                d2 = tlp.tile([P, M], BF16, tag="tl", name=f"d2{it}")
                d3 = tlp.tile([P, M], BF16, tag="tl", name=f"d3{it}")
                g = tlp.tile([P, M], BF16, tag="tl", name=f"g{it}")
                nc.vector.tensor_mul(d1[:, :], u1[:, 0:M], u1[:, M : 2 * M])
                nc.vector.tensor_mul(d2[:, :], u2[:, 0:M], u2[:, M : 2 * M])
                nc.vector.tensor_mul(d3[:, :], u3[:, 0:M], u3[:, M : 2 * M])
                nc.vector.tensor_mul(g[:, :], n2[:, 0:M], n2[:, M : 2 * M])
                dd = tlp.tile([P, M], BF16, tag="tl", name=f"dd{it}")
                d = tlp.tile([P, M], BF16, tag="tl", name=f"d{it}")
                nc.vector.tensor_add(dd[:, :], d1[:, :], d2[:, :])
                nc.vector.tensor_add(d[:, :], dd[:, :], d3[:, :])

                m1 = tlp.tile([P, M], BF16, tag="tl", name=f"m1{it}")
                m2r = tlp.tile([P, M], BF16, tag="tl", name=f"m2r{it}")
                nc.vector._custom_dve(M1C, out=m1[:, :], in0=g[:, :], in1=d[:, :], s0=TINY)
                nc.vector._custom_dve(
                    M2R, out=m2r[:, :], in0=g[:, :], in1=d[:, :],
                    s0=rc["s0"], s1=rc["s1"],
                )
                rho = tlp.tile([P, M], BF16, tag="tl", name=f"rho{it}")
                nc.vector.tensor_mul(rho[:, :], m1[:, :], m2r[:, :])
                t = tlp.tile([P, M], BF16, tag="tl", name=f"t{it}")
                nc.scalar.activation(t[:, :], rho[:, :], AF.Sqrt)

                phi = tlp.tile([P, M], BF16, tag="tl", name=f"phi{it}")
                nc.vector._custom_dve(
                    ATAN7, out=phi[:, :], in0=t[:, :], s0=A1, s1=A2, imm2=A3
                )
                junk = tlp.tile([P, M], BF16, tag="tl", name=f"jk{it}")
                nc.vector._custom_dve(
                    THETA2,
                    out=junk[:, :],
                    accum_out=stats[:, it : it + 1],
                    in0=d[:, :],
                    in1=phi[:, :],
                    s0=PI * PI / 4.0,
                    s1=PI,
                )

            nc.sync.dma_start(out=out[:, :], in_=stats[:, :])
    return nc


_CACHE = {}


def _get_nc(reps=1):
    key = ("nc", reps)
    if key not in _CACHE:
        nc = _emit(bacc.Bacc(), reps=reps)
        nc.compile()
        _CACHE[key] = nc
    return _CACHE[key]


def _pack(pred_sl, targ_sl):
    """[QPC,4] f32 x2 -> [P, QPP*8]: per-iter blocks [w_p w_t x_p x_t ...]."""
    pl = np.empty((P, NIT, 4, 2, M), np.float32)
    pl[:, :, :, 0, :] = pred_sl.reshape(P, NIT, M, 4).transpose(0, 1, 3, 2)
    pl[:, :, :, 1, :] = targ_sl.reshape(P, NIT, M, 4).transpose(0, 1, 3, 2)
    return pl.reshape(P, QPP * 8)


def make_in_maps(pred, target):
    in_maps = []
    for c in range(NCORES):
        sl = slice(c * QPC, (c + 1) * QPC)
        in_maps.append({"pt": _pack(pred[sl], target[sl])})
    return in_maps


def kernel(pred: np.ndarray, target: np.ndarray) -> np.ndarray:
    pred = np.ascontiguousarray(pred, dtype=np.float32)
    target = np.ascontiguousarray(target, dtype=np.float32)
    assert pred.shape == (B, 4) and target.shape == (B, 4)

    nc = _get_nc()
    in_maps = make_in_maps(pred, target)
    res = run_bass_kernel_spmd(nc, in_maps, list(range(NCORES)))
    total = 0.0
    for r in res.results:
        o = np.asarray(r["out"], np.float64)
        total += 4.0 * o.sum()
    return np.float32(total / B)


# revision 6
# speedup vs baseline: 1.0116x; 1.0116x over previous
"""DirectionalRotationLoss Trainium2 kernel (8-core data-parallel), v3.

Math (per quaternion pair p, t — both unnormalized, q = (w,x,y,z)):
  With s1 = (w^2+z^2)/2, s2 = (x^2+y^2)/2, u3 = s1-s2, n2 = s1+s2,
  u1 = wy+xz, u2 = yz-wx the vector u = (u1,u2,u3) satisfies |u| = n2 and
  u/n2 = R(q_hat) e3 up to a fixed diagonal scaling that cancels between
  pred and target.  d = <u_p, u_t>, g = n2_p * n2_t.
  theta = 2*atan(sqrt((g-d)/(g+d))) — tangent half-angle, no arccos.
  With m1 = max(min(g-d, g+d), tiny), m2 = max(g-d, g+d),
  phi = atan(sqrt(m1/m2)):  theta^2/4 = phi^2 + [d<0]*(pi^2/4 - pi*phi).
  Mean over all pairs (x4 applied on host).

Layout: the host packs BOTH tensors into one DRAM param per core with
  per-iteration blocks [w_p w_t | x_p x_t | y_p y_t | z_p z_t], each [P, M].
  One SWDGE cast-DMA per iteration (8 MiB, f32 read -> bf16 in SBUF).
  Component-pair-major order makes every batched operand below a contiguous
  slice, so all stock tensor_tensor ops run in 2x bf16 mode.

Engine split per iteration (tile TT = [P, 8M] bf16):
  GpSimd:  cast-DMA; yz/wx product pair (POOL_OFFLOAD).
  VectorE: products/adds in batched 2M/4M instructions; custom DVE ops
           M1CLAMP, MAXRECIP, ATAN_ODD7, THETA2ACC (theta^2/4 + accumulate).
  ScalarE: 4 in-place Squares over TT (scale=1/sqrt2) after the products
           have consumed TT, plus the Sqrt — one ACT table set.
"""

import numpy as np
from operator import add as _op_add

import concourse.bass as bass
import concourse.bacc as bacc
import concourse.mybir as mybir
from concourse.tile import TileContext
from concourse.bass_utils import run_bass_kernel_spmd
from concourse.dve_spec import (
    Spec,
    Src0,
    Src1,
    C0,
    C1,
    C2,
    Zero,
    One,
    lower,
    sq,
    _has_src1,
    Bin,
    AluOp as _AluOp,
    maxx as _maxx,
    minn as _minn,
)
from concourse.dve_uop import DveOpSpec
import concourse.dve_ops as dve_ops
from concourse.dve_ops import (
    DveOp,
    OPS,
    get_dve_sub_opcode,
    RECIP_APPROX_FAST_CONSTS,
)

NCORES = 8
P = 128
B = 8388608
QPC = B // NCORES          # quats per core
QPP = QPC // P             # quats per partition (8192)
M = 2048                   # quats per partition per iteration
NIT = QPP // M             # iterations (4)

F32 = mybir.dt.float32
BF16 = mybir.dt.bfloat16
AL = mybir.AluOpType
AF = mybir.ActivationFunctionType

INV_SQRT2 = float(1.0 / np.sqrt(2.0))
TINY = 1e-6
PI = float(np.pi)

# 2 -> yz & wx product instructions run on GpSimd instead of VectorE.
POOL_OFFLOAD = 0

# atan(t) ~= t*(1 + t^2*(A1 + A2 t^2 + A3 t^4)) on [0,1]; max err ~8.8e-4 rad.
A1, A2, A3 = -0.33215468080413524, 0.17587160443286803, -0.059202957570287394


def _make_op(name, spec, subdim=False):
    for op in OPS:
        if op.name == name:
            return op
    shas = {}
    op = DveOp(name, spec, subdim=subdim, uops_sha=shas)
    OPS.append(op)
    dve_ops.CUSTOM_DVE_SPECS[name] = spec
    dve_ops._SUB_OPCODE_FOR_NAME[name] = dve_ops._CUSTOM_DVE_ROW_BASE + len(OPS) - 1
    for ver in ("v3", "v4"):
        r = DveOpSpec(
            name=name,
            opcode=get_dve_sub_opcode(name),
            uops=lower(spec, ver=ver),
            rd1_en=_has_src1(spec),
        )
        shas[ver] = r.sha(ver)
    return op


_t2 = sq(Src0)
ATAN7 = _make_op(
    "ATAN_ODD7_ANT",
    Spec(
        body=(((C2 * _t2 + C1) * _t2 + C0) * _t2 + One) * Src0,
        reference=lambda in0, in1, s0, s1, imm2: (
            ((imm2 * in0 * in0 + s1) * in0 * in0 + s0) * in0 * in0 + 1.0
        )
        * in0,
    ),
)

_sub = Src0 - Src1
_add2 = Src0 + Src1
M1C = _make_op(
    "M1CLAMP_ANT",
    Spec(
        body=_maxx(_minn(_sub, _add2), C0),
        reference=lambda in0, in1, s0, s1, imm2: np.maximum(
            np.minimum(in0 - in1, in0 + in1), s0
        ),
    ),
)

# MAXRECIP: y ~= 1/max(in0-in1, in0+in1) via bitwise-NOT seed + 1 Newton step.
_m2 = _maxx(_sub, _add2)
_nt = Bin(_AluOp.BITWISE_NOT, _m2, _m2)
_y0 = _nt * C0
_y1 = _y0 * (C1 - _m2 * _y0)
M2R = _make_op(
    "MAXRECIP_ANT",
    Spec(
        body=_y1,
        reference=lambda in0, in1, s0, s1, imm2: (
            lambda m: (lambda y0: y0 * (s1 - m * y0))(
                (~m.view(np.int32)).view(np.float32) * s0
            )
        )(np.maximum(in0 - in1, in0 + in1)),
    ),
)

# THETA2ACC: out = phi^2 + [d<0]*(C0 - C1*phi); accum_out = sum over free dim.
THETA2 = _make_op(
    "THETA2ACC_ANT",
    Spec(
        body=sq(Src1) + (Src0 < Zero) * (C0 - C1 * Src1),
        accum=_op_add,
        accum_init=Zero,
        reference=lambda in0, in1, s0, s1, imm2: in1 * in1
        + (in0 < 0) * (s0 - s1 * in1),
    ),
)


def _emit(nc, reps=1):
    pt = nc.declare_dram_parameter("pt", [P, QPP * 8], F32, isOutput=False)
    out = nc.declare_dram_parameter("out", [P, NIT], F32, isOutput=True)
    rc = RECIP_APPROX_FAST_CONSTS

    with TileContext(nc) as tc:
        with (
            tc.tile_pool(name="stg", bufs=2) as stg,
            tc.tile_pool(name="pp", bufs=1) as ppp,
            tc.tile_pool(name="w2", bufs=6) as w2p,
            tc.tile_pool(name="u", bufs=6) as up,
            tc.tile_pool(name="tl", bufs=7) as tlp,
            tc.tile_pool(name="st", bufs=1) as stp,
        ):
            stats = stp.tile([P, NIT], F32, tag="st", name="stats")

            import contextlib

            loop_cm = tc.For_i(0, reps, 1) if reps > 1 else contextlib.nullcontext()
            with loop_cm:
              for it in range(NIT):
                TT = stg.tile([P, 8 * M], BF16, tag="stage", name=f"TT{it}")
                nc.gpsimd.dma_start(
                    out=TT[:, :], in_=pt[:, it * 8 * M : (it + 1) * 8 * M]
                )
                # block views (each [P, 2M], both sides side-by-side)
                wb = TT[:, 0 : 2 * M]
                xb = TT[:, 2 * M : 4 * M]
                yb = TT[:, 4 * M : 6 * M]
                zb = TT[:, 6 * M : 8 * M]

                # products first (they read raw TT)
                pp = ppp.tile([P, 4 * M], BF16, tag="pp", name=f"pp{it}")
                nc.vector.tensor_mul(pp[:, :], TT[:, 0 : 4 * M], TT[:, 4 * M : 8 * M])
                yz = w2p.tile([P, 2 * M], BF16, tag="w2", name=f"yz{it}")
                wx = w2p.tile([P, 2 * M], BF16, tag="w2", name=f"wx{it}")
                peng = nc.gpsimd if POOL_OFFLOAD >= 2 else nc.vector
                peng.tensor_mul(yz[:, :], yb, zb)
                peng.tensor_mul(wx[:, :], wb, xb)

                u1 = up.tile([P, 2 * M], BF16, tag="u", name=f"u1{it}")
                u2 = up.tile([P, 2 * M], BF16, tag="u", name=f"u2{it}")
                nc.vector.tensor_add(u1[:, :], pp[:, 0 : 2 * M], pp[:, 2 * M : 4 * M])
                nc.vector.tensor_sub(u2[:, :], yz[:, :], wx[:, :])

                # in-place squares (scale folds the 1/2): TT <- (TT/sqrt2)^2
                for blk in (wb, xb, yb, zb):
                    nc.scalar.activation(blk, blk, AF.Square, scale=INV_SQRT2)

                s1 = w2p.tile([P, 2 * M], BF16, tag="w2", name=f"s1{it}")
                s2 = w2p.tile([P, 2 * M], BF16, tag="w2", name=f"s2{it}")
                nc.vector.tensor_add(s1[:, :], wb, zb)
                nc.vector.tensor_add(s2[:, :], xb, yb)
                u3 = up.tile([P, 2 * M], BF16, tag="u", name=f"u3{it}")
                n2 = up.tile([P, 2 * M], BF16, tag="u", name=f"n2{it}")
                nc.vector.tensor_sub(u3[:, :], s1[:, :], s2[:, :])
                nc.vector.tensor_add(n2[:, :], s1[:, :], s2[:, :])

                # combine p/t halves
                d1 = tlp.tile([P, M], BF16, tag="tl", name=f"d1{it}")
                d2 = tlp.tile([P, M], BF16, tag="tl", name=f"d2{it}")
                d3 = tlp.tile([P, M], BF16, tag="tl", name=f"d3{it}")
                g = tlp.tile([P, M], BF16, tag="tl", name=f"g{it}")
                nc.vector.tensor_mul(d1[:, :], u1[:, 0:M], u1[:, M : 2 * M])
                nc.vector.tensor_mul(d2[:, :], u2[:, 0:M], u2[:, M : 2 * M])
                nc.vector.tensor_mul(d3[:, :], u3[:, 0:M], u3[:, M : 2 * M])
                nc.vector.tensor_mul(g[:, :], n2[:, 0:M], n2[:, M : 2 * M])
                dd = tlp.tile([P, M], BF16, tag="tl", name=f"dd{it}")
                d = tlp.tile([P, M], BF16, tag="tl", name=f"d{it}")
                nc.vector.tensor_add(dd[:, :], d1[:, :], d2[:, :])
                nc.vector.tensor_add(d[:, :], dd[:, :], d3[:, :])

                m1 = tlp.tile([P, M], BF16, tag="tl", name=f"m1{it}")
                m2r = tlp.tile([P, M], BF16, tag="tl", name=f"m2r{it}")
                nc.vector._custom_dve(M1C, out=m1[:, :], in0=g[:, :], in1=d[:, :], s0=TINY)
                nc.vector._custom_dve(
                    M2R, out=m2r[:, :], in0=g[:, :], in1=d[:, :],
                    s0=rc["s0"], s1=rc["s1"],
                )
                rho = tlp.tile([P, M], BF16, tag="tl", name=f"rho{it}")
                nc.vector.tensor_mul(rho[:, :], m1[:, :], m2r[:, :])
                t = tlp.tile([P, M], BF16, tag="tl", name=f"t{it}")
                nc.scalar.activation(t[:, :], rho[:, :], AF.Sqrt)

                phi = tlp.tile([P, M], BF16, tag="tl", name=f"phi{it}")
                nc.vector._custom_dve(
                    ATAN7, out=phi[:, :], in0=t[:, :], s0=A1, s1=A2, imm2=A3
                )
                junk = tlp.tile([P, M], BF16, tag="tl", name=f"jk{it}")
                nc.vector._custom_dve(
                    THETA2,
                    out=junk[:, :],
                    accum_out=stats[:, it : it + 1],
                    in0=d[:, :],
                    in1=phi[:, :],
                    s0=PI * PI / 4.0,
                    s1=PI,
                )

            nc.sync.dma_start(out=out[:, :], in_=stats[:, :])
    return nc


_CACHE = {}


def _get_nc(reps=1):
    key = ("nc", reps)
    if key not in _CACHE:
        nc = _emit(bacc.Bacc(), reps=reps)
        nc.compile()
        _CACHE[key] = nc
    return _CACHE[key]


def _pack(pred_sl, targ_sl):
    """[QPC,4] f32 x2 -> [P, QPP*8]: per-iter blocks [w_p w_t x_p x_t ...]."""
    pl = np.empty((P, NIT, 4, 2, M), np.float32)
    pl[:, :, :, 0, :] = pred_sl.reshape(P, NIT, M, 4).transpose(0, 1, 3, 2)
    pl[:, :, :, 1, :] = targ_sl.reshape(P, NIT, M, 4).transpose(0, 1, 3, 2)
    return pl.reshape(P, QPP * 8)


def make_in_maps(pred, target):
    in_maps = []
    for c in range(NCORES):
        sl = slice(c * QPC, (c + 1) * QPC)
        in_maps.append({"pt": _pack(pred[sl], target[sl])})
    return in_maps


def kernel(pred: np.ndarray, target: np.ndarray) -> np.ndarray:
    pred = np.ascontiguousarray(pred, dtype=np.float32)
    target = np.ascontiguousarray(target, dtype=np.float32)
    assert pred.shape == (B, 4) and target.shape == (B, 4)

    nc = _get_nc()
    in_maps = make_in_maps(pred, target)
    res = run_bass_kernel_spmd(nc, in_maps, list(range(NCORES)))
    total = 0.0
    for r in res.results:
        o = np.asarray(r["out"], np.float64)
        total += 4.0 * o.sum()
    return np.float32(total / B)
